# revision 29
# baseline (speedup 1.0000x reference)
"""MoE layer (top-2 of 8 experts) on 8 trn2 NeuronCores, expert-parallel.

Strategy (per the expert-parallel sharding hint):
  - Host computes the tiny gating network (x @ Wg + bg, softmax, top-2) in
    float64 numpy — 0.27 GFLOP of the 137 GFLOP total — and dispatches
    tokens by expert id: core e receives the tokens routed to expert e,
    pre-scaled by their gate weight and laid out transposed for the PE.
  - Each core runs a pure tiled matmul  y_e = xt_e.T @ We[e]  over its
    [C, 2048] packed token block in float32r (1 cycle/row on the PE vs 4
    for plain fp32).  Weights for expert e are loaded by exactly one core.
  - Host scatters the per-expert outputs back: slot-0 rows are a plain
    assignment (they partition the token set), slot-1 rows are an add.
    The be bias term (w0*be[e0] + w1*be[e1] per token) is added on host.

This does 4x less matmul work than dense dispatch (top-2 of 8 experts) and
is compute-bound: ~2176*2048*2048*2 = 18.3 GFLOP per core.
"""

import numpy as np

N_CORES = 8
N, D, H, E = 8192, 2048, 2048, 8
TOP_K = 2
KT = D // 128  # 16 contraction tiles
HT = H // 512  # 4 output column chunks
C_DEFAULT = 2176  # per-expert token capacity (17 * 128); balanced routing
                  # of 2*8192/8 = 2048 avg tokens/expert leaves ~6% slack

_program_cache: dict[tuple[int, int], object] = {}


def build_program(C: int, repeat: int = 1, loop_repeat: int = 1,
                  B: int = 5, xt_bufs: int = 6, out_bufs: int = 4,
                  we_first: bool = False, split_first: bool = False,
                  out_scalar: bool = False, xt_scalar: bool = False,
                  split_m0: bool = False, no_dma: bool = False,
                  no_pe: bool = False, psum_direct: bool = False,
                  we_big: bool = True, use_bf16: bool = False):
    """SPMD program for one core: y[C, H] = xt[., ., ., .].T @ we[D, H].

    xt layout is [C//128, 128, KT, 128] with xt[m, p, k, j] holding
    (w * x[token])[128*m + j, 128*k + p] so that each m-tile is one
    contiguous 1 MB DMA and xt[m][:, k, :] is directly the [K=128, M=128]
    stationary operand of the PE matmul.
    """
    import concourse.tile as tile
    from concourse import bacc, mybir

    f32 = mybir.dt.float32
    f32r = mybir.dt.bfloat16 if use_bf16 else mybir.dt.float32r
    MT = C // 128

    nc = bacc.Bacc("TRN2", target_bir_lowering=False, debug=False,
                   num_devices=N_CORES)
    xt = nc.declare_dram_parameter("xt", [MT, 128, KT, 128], f32r,
                                   isOutput=False)
    we = nc.declare_dram_parameter("we", [D, H], f32r, isOutput=False)
    y = nc.declare_dram_parameter("y", [C, H], f32, isOutput=True)

    with tile.TileContext(nc) as tc:
        with (
            tc.tile_pool(name="wp", bufs=1) as wp,
            tc.tile_pool(name="xp", bufs=xt_bufs) as xp,
            tc.tile_pool(name="op", bufs=out_bufs) as op,
            tc.tile_pool(name="ps", bufs=8, space="PSUM") as ps,
        ):
            def body():
                # Block-0 xt tiles and the h=0 weight chunk are DMA'd first
                # so the PE can start early; remaining weights stream h-major
                # and block 0's h-phases unlock progressively while the rest
                # of the 16.8 MB preload streams in.
                we_sb = {}

                def emit_we(h):
                    if we_big:
                        if h % 2 == 1:
                            return  # loaded with the even h
                        for k in range(KT):
                            t = wp.tile([128, 1024], f32r, tag=f"web{k}_{h}",
                                        name=f"web_{k}_{h}")
                            nc.sync.dma_start(
                                t[:],
                                we[128 * k:128 * (k + 1),
                                   512 * h:512 * (h + 2)]
                            )
                            we_sb[(k, h)] = t[:, :512]
                            we_sb[(k, h + 1)] = t[:, 512:]
                        return
                    for k in range(KT):
                        t = wp.tile([128, 512], f32r, tag=f"we{k}_{h}",
                                    name=f"we_{k}_{h}")
                        if no_dma:
                            nc.gpsimd.memset(t[:].bitcast(f32), 0.0)
                        else:
                            nc.sync.dma_start(
                                t[:],
                                we[128 * k:128 * (k + 1),
                                   512 * h:512 * (h + 1)]
                            )
                        we_sb[(k, h)] = t

                def load_xt(m):
                    t = xp.tile([128, KT, 128], f32r, tag="xt", name="xt_t")
                    if no_dma:
                        nc.gpsimd.memset(t[:].bitcast(f32), 0.0)
                        return t
                    eng = nc.scalar if xt_scalar else nc.sync
                    eng.dma_start(t[:], xt[m])
                    return t

                if we_first:
                    emit_we(0)
                xt_first = [load_xt(m) for m in range(min(B, MT))]
                if not we_first:
                    emit_we(0)
                for h in range(1, HT):
                    emit_we(h)

                for b0 in range(0, MT, B):
                    blk = range(b0, min(b0 + B, MT))
                    if b0 == 0:
                        xt_blk = {m: xt_first[m] for m in blk}
                    else:
                        xt_blk = {m: load_xt(m) for m in blk}
                    for h in range(HT):
                        for m in blk:
                            out_t = None
                            if no_pe or not psum_direct:
                                out_t = op.tile([128, 512], f32, tag="out",
                                                name="out_t")
                            if no_pe:
                                nc.vector.memset(out_t[:], 0.0)
                            else:
                                acc = ps.tile([128, 512], f32, tag="acc",
                                              name="acc_t")
                                for k in range(KT):
                                    nc.tensor.matmul(
                                        acc[:], xt_blk[m][:, k, :],
                                        we_sb[(k, h)][:],
                                        start=(k == 0), stop=(k == KT - 1),
                                    )
                                if not psum_direct:
                                    nc.vector.tensor_copy(out_t[:], acc[:])
                            if not no_dma:
                                out_eng = nc.scalar if out_scalar else nc.sync
                                src = acc if (psum_direct and not no_pe) else out_t
                                out_eng.dma_start(
                                    y[128 * m:128 * (m + 1),
                                      512 * h:512 * (h + 1)],
                                    src[:],
                                )

            if loop_repeat > 1:
                from concourse import mybir as _mb
                body()
                with tc.For_i(0, loop_repeat - 1, 1,
                              hint_engines=(_mb.EngineType.PE,)):
                    body()
            else:
                for r in range(repeat):
                    body()
    nc.compile()
    return nc


def build_program_v2(C: int, loop_repeat: int = 1, xt_bufs: int = 10,
                     out_bufs: int = 6, out_scalar: bool = False):
    """Half-weights schedule: phase A keeps h in {0,1} weight tiles resident
    (8.4 MB) while streaming all xt m-tiles; phase B reloads xt with h in
    {2,3}.  Halving weight residency frees SBUF for deep xt prefetch and
    halves the front-loaded weight-stream bandwidth demand."""
    import concourse.tile as tile
    from concourse import bacc, mybir

    f32 = mybir.dt.float32
    f32r = mybir.dt.bfloat16 if use_bf16 else mybir.dt.float32r
    MT = C // 128

    nc = bacc.Bacc("TRN2", target_bir_lowering=False, debug=False,
                   num_devices=N_CORES)
    xt = nc.declare_dram_parameter("xt", [MT, 128, KT, 128], f32r,
                                   isOutput=False)
    we = nc.declare_dram_parameter("we", [D, H], f32r, isOutput=False)
    y = nc.declare_dram_parameter("y", [C, H], f32, isOutput=True)

    with tile.TileContext(nc) as tc:
        with (
            tc.tile_pool(name="wp", bufs=1) as wp,
            tc.tile_pool(name="xp", bufs=xt_bufs) as xp,
            tc.tile_pool(name="op", bufs=out_bufs) as op,
            tc.tile_pool(name="ps", bufs=8, space="PSUM") as ps,
        ):
            def body():
                for half in (0, 1):
                    hs = (0, 1) if half == 0 else (2, 3)
                    we_sb = {}

                    def emit_we(h):
                        for k in range(KT):
                            t = wp.tile([128, 512], f32r,
                                        tag=f"we{k}_{h % 2}",
                                        name=f"we_{k}_{h}")
                            nc.sync.dma_start(
                                t[:],
                                we[128 * k:128 * (k + 1),
                                   512 * h:512 * (h + 1)]
                            )
                            we_sb[(k, h)] = t

                    emit_we(hs[0])
                    xt_0 = xp.tile([128, KT, 128], f32r, tag="xt",
                                   name="xt_t")
                    nc.sync.dma_start(xt_0[:], xt[0])
                    emit_we(hs[1])

                    for m in range(MT):
                        if m == 0:
                            xt_m = xt_0
                        else:
                            xt_m = xp.tile([128, KT, 128], f32r, tag="xt",
                                           name="xt_t")
                            nc.sync.dma_start(xt_m[:], xt[m])
                        for h in hs:
                            acc = ps.tile([128, 512], f32, tag="acc",
                                          name="acc_t")
                            for k in range(KT):
                                nc.tensor.matmul(
                                    acc[:], xt_m[:, k, :], we_sb[(k, h)][:],
                                    start=(k == 0), stop=(k == KT - 1),
                                )
                            out_t = op.tile([128, 512], f32, tag="out",
                                            name="out_t")
                            nc.vector.tensor_copy(out_t[:], acc[:])
                            out_eng = nc.scalar if out_scalar else nc.sync
                            out_eng.dma_start(
                                y[128 * m:128 * (m + 1),
                                  512 * h:512 * (h + 1)],
                                out_t[:],
                            )

            if loop_repeat > 1:
                from concourse import mybir as _mb
                body()
                with tc.For_i(0, loop_repeat - 1, 1,
                              hint_engines=(_mb.EngineType.PE,)):
                    body()
            else:
                body()
    nc.compile()
    return nc


def _get_program(C: int, repeat: int = 1, loop_repeat: int = 1, **kw):
    key = (C, repeat, loop_repeat, tuple(sorted(kw.items())))
    if key not in _program_cache:
        _program_cache[key] = build_program(C, repeat, loop_repeat, **kw)
    return _program_cache[key]


def route(x, Wg, bg):
    """Gating + top-2 routing on host (float64 for a stable ordering).

    Returns (e0, e1, w0, w1): per-token top-1/top-2 expert ids and their
    (unnormalized) softmax gate weights, matching jax.lax.top_k tie-break
    (lower index wins).
    """
    logits = x.astype(np.float64) @ Wg.astype(np.float64) + bg.astype(np.float64)
    order = np.argsort(-logits, axis=1, kind="stable")
    e0 = order[:, 0].astype(np.int32)
    e1 = order[:, 1].astype(np.int32)
    mx = logits.max(axis=1, keepdims=True)
    p = np.exp(logits - mx)
    gate = p / p.sum(axis=1, keepdims=True)
    n = np.arange(logits.shape[0])
    w0 = gate[n, e0].astype(np.float32)
    w1 = gate[n, e1].astype(np.float32)
    return e0, e1, w0, w1


def kernel(x, Wg, bg, We, be):
    x = np.ascontiguousarray(np.asarray(x, dtype=np.float32))
    Wg = np.asarray(Wg, dtype=np.float32)
    bg = np.asarray(bg, dtype=np.float32)
    We = np.asarray(We, dtype=np.float32)
    be = np.asarray(be, dtype=np.float32)

    e0, e1, w0, w1 = route(x, Wg, bg)

    # Per-expert token lists: slot-0 tokens first, then slot-1 tokens.
    idx0 = [np.nonzero(e0 == e)[0] for e in range(E)]
    idx1 = [np.nonzero(e1 == e)[0] for e in range(E)]
    counts = [len(idx0[e]) + len(idx1[e]) for e in range(E)]
    cmax = max(counts)
    C = max(C_DEFAULT, ((cmax + 127) // 128) * 128)

    nc = _get_program(C)

    in_maps = []
    for e in range(E):
        idx = np.concatenate([idx0[e], idx1[e]])
        w = np.concatenate([w0[idx0[e]], w1[idx1[e]]])
        xq = np.zeros((C, D), dtype=np.float32)
        xq[:len(idx)] = x[idx] * w[:, None]
        # [C, D] -> [MT, 128, KT, 128] with axes (m, p, k, j)
        a = xq.reshape(C // 128, 128, KT, 128).transpose(0, 3, 2, 1)
        in_maps.append({
            "xt": np.ascontiguousarray(a),
            "we": np.ascontiguousarray(We[e]),
        })

    from concourse.bass_utils import run_bass_kernel_spmd
    res = run_bass_kernel_spmd(nc, in_maps, core_ids=list(range(N_CORES)))

    out = np.empty((N, H), dtype=np.float32)
    for e in range(E):
        y = res.results[e]["y"]
        n0 = len(idx0[e])
        out[idx0[e]] = y[:n0]
    for e in range(E):
        y = res.results[e]["y"]
        n0 = len(idx0[e])
        out[idx1[e]] += y[n0:counts[e]]

    if be.any():
        out += w0[:, None] * be[e0] + w1[:, None] * be[e1]
    return out


# revision 31
# speedup vs baseline: 1.0056x; 1.0056x over previous
"""MoE layer (top-2 of 8 experts) on 8 trn2 NeuronCores, expert-parallel.

Strategy (per the expert-parallel sharding hint):
  - Host computes the tiny gating network (x @ Wg + bg, softmax, top-2) in
    float64 numpy — 0.27 GFLOP of the 137 GFLOP total — and dispatches
    tokens by expert id: core e receives the tokens routed to expert e,
    pre-scaled by their gate weight and laid out transposed for the PE.
  - Each core runs a pure tiled matmul  y_e = xt_e.T @ We[e]  over its
    [C, 2048] packed token block in float32r (1 cycle/row on the PE vs 4
    for plain fp32).  Weights for expert e are loaded by exactly one core.
  - Host scatters the per-expert outputs back: slot-0 rows are a plain
    assignment (they partition the token set), slot-1 rows are an add.
    The be bias term (w0*be[e0] + w1*be[e1] per token) is added on host.

This does 4x less matmul work than dense dispatch (top-2 of 8 experts) and
is compute-bound: ~2176*2048*2048*2 = 18.3 GFLOP per core.
"""

import numpy as np

N_CORES = 8
N, D, H, E = 8192, 2048, 2048, 8
TOP_K = 2
KT = D // 128  # 16 contraction tiles
HT = H // 512  # 4 output column chunks
C_DEFAULT = 2176  # per-expert token capacity (17 * 128); balanced routing
                  # of 2*8192/8 = 2048 avg tokens/expert leaves ~6% slack

_program_cache: dict[tuple[int, int], object] = {}


def build_program(C: int, repeat: int = 1, loop_repeat: int = 1,
                  B: int = 5, xt_bufs: int = 6, out_bufs: int = 4,
                  we_first: bool = False, split_first: bool = False,
                  out_scalar: bool = False, xt_scalar: bool = False,
                  split_m0: bool = False, no_dma: bool = False,
                  no_pe: bool = False, psum_direct: bool = False,
                  we_big: bool = True, use_bf16: bool = False,
                  we_h0_split: bool = False, out_gpsimd: bool = False,
                  xt_gpsimd: bool = False):
    """SPMD program for one core: y[C, H] = xt[., ., ., .].T @ we[D, H].

    xt layout is [C//128, 128, KT, 128] with xt[m, p, k, j] holding
    (w * x[token])[128*m + j, 128*k + p] so that each m-tile is one
    contiguous 1 MB DMA and xt[m][:, k, :] is directly the [K=128, M=128]
    stationary operand of the PE matmul.
    """
    import concourse.tile as tile
    from concourse import bacc, mybir

    f32 = mybir.dt.float32
    f32r = mybir.dt.bfloat16 if use_bf16 else mybir.dt.float32r
    MT = C // 128

    nc = bacc.Bacc("TRN2", target_bir_lowering=False, debug=False,
                   num_devices=N_CORES)
    xt = nc.declare_dram_parameter("xt", [MT, 128, KT, 128], f32r,
                                   isOutput=False)
    we = nc.declare_dram_parameter("we", [D, H], f32r, isOutput=False)
    y = nc.declare_dram_parameter("y", [C, H], f32, isOutput=True)

    with tile.TileContext(nc) as tc:
        with (
            tc.tile_pool(name="wp", bufs=1) as wp,
            tc.tile_pool(name="xp", bufs=xt_bufs) as xp,
            tc.tile_pool(name="op", bufs=out_bufs) as op,
            tc.tile_pool(name="ps", bufs=8, space="PSUM") as ps,
        ):
            def body():
                # Block-0 xt tiles and the h=0 weight chunk are DMA'd first
                # so the PE can start early; remaining weights stream h-major
                # and block 0's h-phases unlock progressively while the rest
                # of the 16.8 MB preload streams in.
                we_sb = {}

                def emit_we(h):
                    if we_big and h == 0 and we_h0_split:
                        # h0 pair as separate 512-wide tiles in one big slot
                        for k in range(KT):
                            t = wp.tile([128, 1024], f32r, tag=f"web{k}_{h}",
                                        name=f"web_{k}_{h}")
                            nc.sync.dma_start(
                                t[:, :512],
                                we[128 * k:128 * (k + 1), 0:512])
                            nc.sync.dma_start(
                                t[:, 512:],
                                we[128 * k:128 * (k + 1), 512:1024])
                            we_sb[(k, 0)] = t[:, :512]
                            we_sb[(k, 1)] = t[:, 512:]
                        return
                    if we_big:
                        if h % 2 == 1:
                            return  # loaded with the even h
                        for k in range(KT):
                            t = wp.tile([128, 1024], f32r, tag=f"web{k}_{h}",
                                        name=f"web_{k}_{h}")
                            nc.sync.dma_start(
                                t[:],
                                we[128 * k:128 * (k + 1),
                                   512 * h:512 * (h + 2)]
                            )
                            we_sb[(k, h)] = t[:, :512]
                            we_sb[(k, h + 1)] = t[:, 512:]
                        return
                    for k in range(KT):
                        t = wp.tile([128, 512], f32r, tag=f"we{k}_{h}",
                                    name=f"we_{k}_{h}")
                        if no_dma:
                            nc.gpsimd.memset(t[:].bitcast(f32), 0.0)
                        else:
                            nc.sync.dma_start(
                                t[:],
                                we[128 * k:128 * (k + 1),
                                   512 * h:512 * (h + 1)]
                            )
                        we_sb[(k, h)] = t

                def load_xt(m):
                    t = xp.tile([128, KT, 128], f32r, tag="xt", name="xt_t")
                    if no_dma:
                        nc.gpsimd.memset(t[:].bitcast(f32), 0.0)
                        return t
                    eng = (nc.gpsimd if xt_gpsimd
                           else nc.scalar if xt_scalar else nc.sync)
                    eng.dma_start(t[:], xt[m])
                    return t

                if we_first:
                    emit_we(0)
                xt_first = [load_xt(m) for m in range(min(B, MT))]
                if not we_first:
                    emit_we(0)
                for h in range(1, HT):
                    if we_big and h == 1:
                        continue
                    emit_we(h)

                for b0 in range(0, MT, B):
                    blk = range(b0, min(b0 + B, MT))
                    if b0 == 0:
                        xt_blk = {m: xt_first[m] for m in blk}
                    else:
                        xt_blk = {m: load_xt(m) for m in blk}
                    for h in range(HT):
                        for m in blk:
                            out_t = None
                            if no_pe or not psum_direct:
                                out_t = op.tile([128, 512], f32, tag="out",
                                                name="out_t")
                            if no_pe:
                                nc.vector.memset(out_t[:], 0.0)
                            else:
                                acc = ps.tile([128, 512], f32, tag="acc",
                                              name="acc_t")
                                for k in range(KT):
                                    nc.tensor.matmul(
                                        acc[:], xt_blk[m][:, k, :],
                                        we_sb[(k, h)][:],
                                        start=(k == 0), stop=(k == KT - 1),
                                    )
                                if not psum_direct:
                                    nc.vector.tensor_copy(out_t[:], acc[:])
                            if not no_dma:
                                out_eng = (nc.gpsimd if out_gpsimd
                                           else nc.scalar if out_scalar
                                           else nc.sync)
                                src = acc if (psum_direct and not no_pe) else out_t
                                out_eng.dma_start(
                                    y[128 * m:128 * (m + 1),
                                      512 * h:512 * (h + 1)],
                                    src[:],
                                )

            if loop_repeat > 1:
                from concourse import mybir as _mb
                body()
                with tc.For_i(0, loop_repeat - 1, 1,
                              hint_engines=(_mb.EngineType.PE,)):
                    body()
            else:
                for r in range(repeat):
                    body()
    nc.compile()
    return nc


def build_program_v2(C: int, loop_repeat: int = 1, xt_bufs: int = 10,
                     out_bufs: int = 6, out_scalar: bool = False):
    """Half-weights schedule: phase A keeps h in {0,1} weight tiles resident
    (8.4 MB) while streaming all xt m-tiles; phase B reloads xt with h in
    {2,3}.  Halving weight residency frees SBUF for deep xt prefetch and
    halves the front-loaded weight-stream bandwidth demand."""
    import concourse.tile as tile
    from concourse import bacc, mybir

    f32 = mybir.dt.float32
    f32r = mybir.dt.bfloat16 if use_bf16 else mybir.dt.float32r
    MT = C // 128

    nc = bacc.Bacc("TRN2", target_bir_lowering=False, debug=False,
                   num_devices=N_CORES)
    xt = nc.declare_dram_parameter("xt", [MT, 128, KT, 128], f32r,
                                   isOutput=False)
    we = nc.declare_dram_parameter("we", [D, H], f32r, isOutput=False)
    y = nc.declare_dram_parameter("y", [C, H], f32, isOutput=True)

    with tile.TileContext(nc) as tc:
        with (
            tc.tile_pool(name="wp", bufs=1) as wp,
            tc.tile_pool(name="xp", bufs=xt_bufs) as xp,
            tc.tile_pool(name="op", bufs=out_bufs) as op,
            tc.tile_pool(name="ps", bufs=8, space="PSUM") as ps,
        ):
            def body():
                for half in (0, 1):
                    hs = (0, 1) if half == 0 else (2, 3)
                    we_sb = {}

                    def emit_we(h):
                        for k in range(KT):
                            t = wp.tile([128, 512], f32r,
                                        tag=f"we{k}_{h % 2}",
                                        name=f"we_{k}_{h}")
                            nc.sync.dma_start(
                                t[:],
                                we[128 * k:128 * (k + 1),
                                   512 * h:512 * (h + 1)]
                            )
                            we_sb[(k, h)] = t

                    emit_we(hs[0])
                    xt_0 = xp.tile([128, KT, 128], f32r, tag="xt",
                                   name="xt_t")
                    nc.sync.dma_start(xt_0[:], xt[0])
                    emit_we(hs[1])

                    for m in range(MT):
                        if m == 0:
                            xt_m = xt_0
                        else:
                            xt_m = xp.tile([128, KT, 128], f32r, tag="xt",
                                           name="xt_t")
                            nc.sync.dma_start(xt_m[:], xt[m])
                        for h in hs:
                            acc = ps.tile([128, 512], f32, tag="acc",
                                          name="acc_t")
                            for k in range(KT):
                                nc.tensor.matmul(
                                    acc[:], xt_m[:, k, :], we_sb[(k, h)][:],
                                    start=(k == 0), stop=(k == KT - 1),
                                )
                            out_t = op.tile([128, 512], f32, tag="out",
                                            name="out_t")
                            nc.vector.tensor_copy(out_t[:], acc[:])
                            out_eng = nc.scalar if out_scalar else nc.sync
                            out_eng.dma_start(
                                y[128 * m:128 * (m + 1),
                                  512 * h:512 * (h + 1)],
                                out_t[:],
                            )

            if loop_repeat > 1:
                from concourse import mybir as _mb
                body()
                with tc.For_i(0, loop_repeat - 1, 1,
                              hint_engines=(_mb.EngineType.PE,)):
                    body()
            else:
                body()
    nc.compile()
    return nc


def _get_program(C: int, repeat: int = 1, loop_repeat: int = 1, **kw):
    key = (C, repeat, loop_repeat, tuple(sorted(kw.items())))
    if key not in _program_cache:
        _program_cache[key] = build_program(C, repeat, loop_repeat, **kw)
    return _program_cache[key]


def route(x, Wg, bg):
    """Gating + top-2 routing on host (float64 for a stable ordering).

    Returns (e0, e1, w0, w1): per-token top-1/top-2 expert ids and their
    (unnormalized) softmax gate weights, matching jax.lax.top_k tie-break
    (lower index wins).
    """
    logits = x.astype(np.float64) @ Wg.astype(np.float64) + bg.astype(np.float64)
    order = np.argsort(-logits, axis=1, kind="stable")
    e0 = order[:, 0].astype(np.int32)
    e1 = order[:, 1].astype(np.int32)
    mx = logits.max(axis=1, keepdims=True)
    p = np.exp(logits - mx)
    gate = p / p.sum(axis=1, keepdims=True)
    n = np.arange(logits.shape[0])
    w0 = gate[n, e0].astype(np.float32)
    w1 = gate[n, e1].astype(np.float32)
    return e0, e1, w0, w1


def kernel(x, Wg, bg, We, be):
    x = np.ascontiguousarray(np.asarray(x, dtype=np.float32))
    Wg = np.asarray(Wg, dtype=np.float32)
    bg = np.asarray(bg, dtype=np.float32)
    We = np.asarray(We, dtype=np.float32)
    be = np.asarray(be, dtype=np.float32)

    e0, e1, w0, w1 = route(x, Wg, bg)

    # Per-expert token lists: slot-0 tokens first, then slot-1 tokens.
    idx0 = [np.nonzero(e0 == e)[0] for e in range(E)]
    idx1 = [np.nonzero(e1 == e)[0] for e in range(E)]
    counts = [len(idx0[e]) + len(idx1[e]) for e in range(E)]
    cmax = max(counts)
    C = max(C_DEFAULT, ((cmax + 127) // 128) * 128)

    nc = _get_program(C)

    in_maps = []
    for e in range(E):
        idx = np.concatenate([idx0[e], idx1[e]])
        w = np.concatenate([w0[idx0[e]], w1[idx1[e]]])
        xq = np.zeros((C, D), dtype=np.float32)
        xq[:len(idx)] = x[idx] * w[:, None]
        # [C, D] -> [MT, 128, KT, 128] with axes (m, p, k, j)
        a = xq.reshape(C // 128, 128, KT, 128).transpose(0, 3, 2, 1)
        in_maps.append({
            "xt": np.ascontiguousarray(a),
            "we": np.ascontiguousarray(We[e]),
        })

    from concourse.bass_utils import run_bass_kernel_spmd
    res = run_bass_kernel_spmd(nc, in_maps, core_ids=list(range(N_CORES)))

    out = np.empty((N, H), dtype=np.float32)
    for e in range(E):
        y = res.results[e]["y"]
        n0 = len(idx0[e])
        out[idx0[e]] = y[:n0]
    for e in range(E):
        y = res.results[e]["y"]
        n0 = len(idx0[e])
        out[idx1[e]] += y[n0:counts[e]]

    if be.any():
        out += w0[:, None] * be[e0] + w1[:, None] * be[e1]
    return out
